# revision 1
# baseline (speedup 1.0000x reference)
"""Trainium2 Bass kernel for EquivariantBinaryClassificationNoGraphScalar.

Computation (see reference):
    s[b, c]  = sum_n x[b, n, c]                      # node-sum, N=256
    h        = LayerNorm_C(s) * ln_w + ln_b          # over C=1024
    out[b]   = sigmoid(h . W[0] + b)                 # Linear(C, 1)

Sharding: data-parallel over batch. x is [1024, 256, 1024] f32 (1 GiB);
each of 8 cores gets a [128, 256, 1024] shard (128 MiB) -> memory-bound,
per-core HBM roofline ~128MiB / 358GB/s ~= 375 us.

Per-core algorithm (batch lives on the partition axis, so no
cross-partition reduction is ever needed):
  - Stream x as [128(batch), NCHUNK(node), 1024(chan)] tiles: partition
    stride 1 MiB, NCHUNK*4KiB contiguous per partition.
  - VectorE accumulates acc[b, c] += x[:, n, :] (one 1x-mode f32 add per
    node slice; ~256 * 1024 cycles ~= 190 us, hidden under DMA).
  - Epilogue for all 128 batches at once: bn_stats/bn_aggr -> mu, var;
    rstd = 1/sqrt(var+eps); logits = rstd*(s.wln - mu*sum(wln)) + c0
    with wln = ln_w*W[0], c0 = sum(ln_b*W[0]) + b; sigmoid on ScalarE.
"""

import sys

import numpy as np

if "/opt/trn_rl_repo" not in sys.path:
    sys.path.insert(0, "/opt/trn_rl_repo")

from contextlib import ExitStack

import concourse.bacc as bacc
import concourse.bass as bass
import concourse.tile as tile
from concourse import mybir
from concourse.bass_utils import run_bass_kernel_spmd

B, N, C = 1024, 256, 1024
NCORES = 8
BS = B // NCORES  # 128 batches per core
P = 128
FP32 = mybir.dt.float32
LN_EPS = 1e-5

NCHUNK = 4  # node slices per DMA -> 2 MiB per transfer
X_BUFS = 6

# Kept for test.py: the BassKernelResults of the last kernel() call
# (exec_time_ns is populated when BASS_TRACE=1).
LAST_RESULT = None


def build(bs: int = BS, nchunk: int = NCHUNK, x_bufs: int = X_BUFS, passes: int = 1):
    """Build the per-core Bass module. bs<128 gives a small variant for sim.

    passes>1 streams x that many times (acc reset each pass; result
    unchanged) — used by test.py to measure pure device time per pass as
    slope(passes=2) - slope(passes=1).
    """
    # Bacc (not raw Bass): its finalize() runs generate_event_semaphores,
    # which splits multi-sem waits (TRN2 allows 1 sync wait per instruction).
    nc = bacc.Bacc(None)
    x = nc.declare_dram_parameter("x", [bs, N, C], FP32, isOutput=False)
    ln_w = nc.declare_dram_parameter("ln_w", [C], FP32, isOutput=False)
    ln_b = nc.declare_dram_parameter("ln_b", [C], FP32, isOutput=False)
    W = nc.declare_dram_parameter("W", [1, C], FP32, isOutput=False)
    bias = nc.declare_dram_parameter("b", [1], FP32, isOutput=False)
    out = nc.declare_dram_parameter("out", [bs, 1], FP32, isOutput=True)

    with tile.TileContext(nc) as tc, ExitStack() as ctx:
        xpool = ctx.enter_context(tc.tile_pool(name="xp", bufs=x_bufs))
        singles = ctx.enter_context(tc.tile_pool(name="si", bufs=1))
        ep = ctx.enter_context(tc.tile_pool(name="ep", bufs=1))

        eps_t = singles.tile([P, 1], FP32)
        nc.vector.memset(eps_t, LN_EPS)

        def bcast_load(src_ap, ncols, name):
            """Replicate a [ncols] DRAM vector across all partitions."""
            t = singles.tile([P, ncols], FP32, name=name)
            bc = bass.AP(
                tensor=src_ap.tensor,
                offset=src_ap.offset,
                ap=[[0, P]] + [list(d) for d in src_ap.ap],
            )
            nc.gpsimd.dma_start(out=t, in_=bc)
            return t

        lnw_t = bcast_load(ln_w[:], C, "lnw_t")
        lnb_t = bcast_load(ln_b[:], C, "lnb_t")
        w_t = bcast_load(W[0], C, "w_t")
        b_t = bcast_load(bias[:], 1, "b_t")

        # ---- main loop: acc[b, c] = sum_n x[b, n, c] ----
        acc = singles.tile([P, C], FP32)
        for _ in range(passes):
            nc.vector.memset(acc[:bs], 0.0)
            for n0 in range(0, N, nchunk):
                xt = xpool.tile([P, nchunk, C], FP32)
                nc.sync.dma_start(out=xt[:bs], in_=x[:, n0 : n0 + nchunk, :])
                for j in range(nchunk):
                    nc.vector.tensor_add(acc[:bs], acc[:bs], xt[:bs, j, :])

        # ---- epilogue: all `bs` batches at once, partition = batch ----
        s = acc
        stats = ep.tile([P, 2, 6], FP32)
        sv = s.rearrange("p (g d) -> p g d", g=2)
        for g in range(2):
            nc.vector.bn_stats(out=stats[:bs, g, :], in_=sv[:bs, g, :])
        mv = ep.tile([P, 2], FP32)
        nc.vector.bn_aggr(out=mv[:bs], in_=stats[:bs])
        mu = mv[:bs, 0:1]
        var = mv[:bs, 1:2]

        std = ep.tile([P, 1], FP32)
        nc.scalar.activation(
            out=std[:bs],
            in_=var,
            func=mybir.ActivationFunctionType.Sqrt,
            bias=eps_t[:bs],
            scale=1.0,
        )
        rstd = ep.tile([P, 1], FP32)
        nc.vector.reciprocal(out=rstd[:bs], in_=std[:bs])

        # wln = ln_w * W ; swln = sum(wln) ; c0 = sum(ln_b * W) + b
        # (DVE instructions encode at most ONE sync wait, so give each
        # broadcast-DMA'd tile a single-dependency first consumer.)
        wcopy = singles.tile([P, C], FP32)
        nc.vector.tensor_copy(wcopy, w_t)
        wln = singles.tile([P, C], FP32)
        nc.vector.tensor_mul(wln, lnw_t, wcopy)
        swln = ep.tile([P, 1], FP32)
        nc.vector.reduce_sum(out=swln, in_=wln, axis=mybir.AxisListType.X)
        # (tensor_tensor_reduce is avoided: its custom DVE ucode isn't
        # shipped via this compile path and it kills the exec unit.)
        scr0 = ep.tile([P, C], FP32)
        c0 = ep.tile([P, 1], FP32)
        nc.vector.tensor_mul(scr0, lnb_t, wcopy)
        nc.vector.reduce_sum(out=c0, in_=scr0, axis=mybir.AxisListType.X)
        nc.vector.tensor_add(c0, c0, b_t)

        # dot = s . wln  (per batch row)
        scr1 = ep.tile([P, C], FP32)
        dot = ep.tile([P, 1], FP32)
        nc.vector.tensor_mul(scr1[:bs], s[:bs], wln[:bs])
        nc.vector.reduce_sum(out=dot[:bs], in_=scr1[:bs], axis=mybir.AxisListType.X)

        # logits = rstd * (dot - mu * swln); out = sigmoid(logits + c0)
        t0 = ep.tile([P, 1], FP32)
        nc.vector.tensor_mul(t0[:bs], mu, swln[:bs])
        t1 = ep.tile([P, 1], FP32)
        nc.vector.tensor_sub(t1[:bs], dot[:bs], t0[:bs])
        t2 = ep.tile([P, 1], FP32)
        nc.vector.tensor_mul(t2[:bs], t1[:bs], rstd[:bs])
        res = ep.tile([P, 1], FP32)
        nc.scalar.activation(
            out=res[:bs],
            in_=t2[:bs],
            func=mybir.ActivationFunctionType.Sigmoid,
            bias=c0[:bs],
            scale=1.0,
        )
        nc.sync.dma_start(out=out[:, :], in_=res[:bs])

    # Run the Bacc compile pipeline (register allocation + multi-sync-wait
    # splitting via generate_event_semaphores) — nothing else in the
    # run_bass_kernel_spmd/axon path calls finalize for us.
    nc.finalize()
    return nc


_NC_CACHE = {}


def kernel(**inputs) -> np.ndarray:
    global LAST_RESULT
    x = np.ascontiguousarray(np.asarray(inputs["x"], dtype=np.float32))
    ln_w = np.ascontiguousarray(np.asarray(inputs["ln_w"], dtype=np.float32))
    ln_b = np.ascontiguousarray(np.asarray(inputs["ln_b"], dtype=np.float32))
    W = np.ascontiguousarray(np.asarray(inputs["W"], dtype=np.float32))
    b = np.ascontiguousarray(np.asarray(inputs["b"], dtype=np.float32))

    if "full" not in _NC_CACHE:
        _NC_CACHE["full"] = build()
    nc = _NC_CACHE["full"]

    in_maps = [
        {
            "x": x[i * BS : (i + 1) * BS],
            "ln_w": ln_w,
            "ln_b": ln_b,
            "W": W,
            "b": b,
        }
        for i in range(NCORES)
    ]
    res = run_bass_kernel_spmd(nc, in_maps, list(range(NCORES)))
    LAST_RESULT = res
    return np.concatenate([res.results[i]["out"] for i in range(NCORES)], axis=0)



# revision 2
# speedup vs baseline: 1.5650x; 1.5650x over previous
"""Trainium2 Bass kernel for EquivariantBinaryClassificationNoGraphScalar.

Computation (see reference):
    s[b, c]  = sum_n x[b, n, c]                      # node-sum, N=256
    h        = LayerNorm_C(s) * ln_w + ln_b          # over C=1024
    out[b]   = sigmoid(h . W[0] + b)                 # Linear(C, 1)

Sharding: data-parallel over batch. x is [1024, 256, 1024] f32 (1 GiB);
each of 8 cores gets a [128, 256, 1024] shard (128 MiB) -> memory-bound,
per-core HBM roofline ~128MiB / 358GB/s ~= 375 us.

Per-core algorithm (v2 — TensorE does the node-sum, DVE stays idle):
  - Stream x in fully-contiguous G-MiB DMAs: x[b0:b0+G] viewed as
    [128(p), 2G*1024] (partition p holds node rows 2G*p..2G*p+2G-1 of
    the G-batch block; batch g sits on partitions [g*128/G,(g+1)*128/G)).
    HWDGE transfers alternate between the SP and ACT rings to hide the
    per-transfer completion gap.
  - fp32r matmuls against a shifted-window mask lhsT [128, 128] whose
    column b0+g is ones exactly on batch g's partition block contract
    over the partition (node) axis at 1 cycle/row: PSUM bank ch
    accumulates s[b, ch*512:(ch+1)*512] with partition = batch.
    (fp32r operands must be produced as fp32r, so x and the mask are
    declared float32r in DRAM — same bits as f32; the mask ships as an
    extra host-provided input because memset can't write fp32r and the
    fp32->fp32r DVE cast-copy ucode kills the exec unit.)
  - One PSUM->SBUF copy at the end, then the LN+Linear+sigmoid
    epilogue: bn_stats/bn_aggr -> mu, var; rstd = 1/sqrt(var+eps);
    logits = rstd*(s.wln - mu*sum(wln)) + c0 with wln = ln_w*W[0],
    c0 = sum(ln_b*W[0]) + b; sigmoid on ScalarE.
"""

import sys

import numpy as np

if "/opt/trn_rl_repo" not in sys.path:
    sys.path.insert(0, "/opt/trn_rl_repo")

from contextlib import ExitStack

import concourse.bacc as bacc
import concourse.bass as bass
import concourse.tile as tile
from concourse import mybir
from concourse.bass_utils import run_bass_kernel_spmd

B, N, C = 1024, 256, 1024
NCORES = 8
BS = B // NCORES  # 128 batches per core
P = 128
FP32 = mybir.dt.float32
FP32R = mybir.dt.float32r
LN_EPS = 1e-5

G_DEFAULT = 2  # batches per DMA -> G MiB contiguous per transfer
BUFS_DEFAULT = 8

# Kept for test.py: the BassKernelResults of the last kernel() call.
LAST_RESULT = None


def make_mask(G: int = G_DEFAULT) -> np.ndarray:
    """Host-side mask constant: cols [128, 128+G) hold the per-batch-block
    ones columns; window [:, 128-b0 : 256-b0] puts block-g ones into lhsT
    column b0+g and zeros elsewhere."""
    PB = P // G
    m = np.zeros((P, 256 + G), np.float32)
    for g in range(G):
        m[g * PB : (g + 1) * PB, 128 + g] = 1.0
    return m


def aux_inputs(G: int = G_DEFAULT) -> dict:
    return {"mask": make_mask(G)}


def build(
    bs: int = BS,
    passes: int = 1,
    bufs: int = BUFS_DEFAULT,
    alt_queues: bool = True,
    G: int = G_DEFAULT,
):
    """Build the per-core Bass module. passes>1 streams x that many times
    (PSUM start=True resets each pass; result unchanged) — used by test.py
    to measure pure device time per pass via the slope method."""
    assert bs % G == 0 and 128 % G == 0
    R = 2 * G  # node rows per partition

    # Bacc (not raw Bass): its finalize() runs generate_event_semaphores,
    # which splits multi-sem waits (TRN2 allows 1 sync wait per instruction).
    nc = bacc.Bacc(None)
    x = nc.declare_dram_parameter("x", [bs, N, C], FP32R, isOutput=False)
    ln_w = nc.declare_dram_parameter("ln_w", [C], FP32, isOutput=False)
    ln_b = nc.declare_dram_parameter("ln_b", [C], FP32, isOutput=False)
    W = nc.declare_dram_parameter("W", [1, C], FP32, isOutput=False)
    bias = nc.declare_dram_parameter("b", [1], FP32, isOutput=False)
    mask = nc.declare_dram_parameter("mask", [P, 256 + G], FP32R, isOutput=False)
    out = nc.declare_dram_parameter("out", [bs, 1], FP32, isOutput=True)

    with tile.TileContext(nc) as tc, ExitStack() as ctx:
        xpool = ctx.enter_context(tc.tile_pool(name="xp", bufs=bufs))
        singles = ctx.enter_context(tc.tile_pool(name="si", bufs=1))
        ep = ctx.enter_context(tc.tile_pool(name="ep", bufs=1))
        psum = ctx.enter_context(tc.tile_pool(name="ps", bufs=1, space="PSUM"))

        eps_t = singles.tile([P, 1], FP32)
        nc.vector.memset(eps_t, LN_EPS)

        maskbuf = singles.tile([P, 256 + G], FP32R)
        nc.sync.dma_start(out=maskbuf, in_=mask[:, :])

        def bcast_load(src_ap, ncols, name):
            """Replicate a [ncols] DRAM vector across all partitions."""
            t = singles.tile([P, ncols], FP32, name=name)
            bc = bass.AP(
                tensor=src_ap.tensor,
                offset=src_ap.offset,
                ap=[[0, P]] + [list(d) for d in src_ap.ap],
            )
            nc.gpsimd.dma_start(out=t, in_=bc)
            return t

        lnw_t = bcast_load(ln_w[:], C, "lnw_t")
        lnb_t = bcast_load(ln_b[:], C, "lnb_t")
        w_t = bcast_load(W[0], C, "w_t")
        b_t = bcast_load(bias[:], 1, "b_t")

        ps = psum.tile([P, 2, 512], FP32)
        acc = singles.tile([P, C], FP32)

        ngroups = bs // G
        for _ in range(passes):
            for gi in range(ngroups):
                b0 = gi * G
                xt = xpool.tile([P, R * C], FP32R)
                src = bass.AP(
                    tensor=x[0].tensor,
                    offset=b0 * N * C,
                    ap=[[R * C, P], [1, R * C]],
                )
                eng = nc.scalar if (alt_queues and gi % 2) else nc.sync
                eng.dma_start(out=xt, in_=src)
                lhsT = maskbuf[:, 128 - b0 : 256 - b0]
                for j in range(R):
                    for ch in (0, 1):
                        rhs = xt[:, j * C + ch * 512 : j * C + (ch + 1) * 512]
                        nc.tensor.matmul(
                            ps[:, ch, :],
                            lhsT,
                            rhs,
                            start=(gi == 0 and j == 0),
                            stop=(gi == ngroups - 1 and j == R - 1),
                            skip_group_check=True,
                        )
            # evacuate PSUM -> SBUF (inside the pass loop so each pass is
            # self-contained; ~1.2us, negligible)
            nc.vector.tensor_copy(acc[:bs], ps[:bs].rearrange("p a b -> p (a b)"))

        # ---- epilogue: all `bs` batches at once, partition = batch ----
        s = acc
        stats = ep.tile([P, 2, 6], FP32)
        sv = s.rearrange("p (g d) -> p g d", g=2)
        for g in range(2):
            nc.vector.bn_stats(out=stats[:bs, g, :], in_=sv[:bs, g, :])
        mv = ep.tile([P, 2], FP32)
        nc.vector.bn_aggr(out=mv[:bs], in_=stats[:bs])
        mu = mv[:bs, 0:1]
        var = mv[:bs, 1:2]

        std = ep.tile([P, 1], FP32)
        nc.scalar.activation(
            out=std[:bs],
            in_=var,
            func=mybir.ActivationFunctionType.Sqrt,
            bias=eps_t[:bs],
            scale=1.0,
        )
        rstd = ep.tile([P, 1], FP32)
        nc.vector.reciprocal(out=rstd[:bs], in_=std[:bs])

        # wln = ln_w * W ; swln = sum(wln) ; c0 = sum(ln_b * W) + b
        # (DVE instructions encode at most ONE sync wait, so give each
        # broadcast-DMA'd tile a single-dependency first consumer.)
        wcopy = singles.tile([P, C], FP32)
        nc.vector.tensor_copy(wcopy, w_t)
        wln = singles.tile([P, C], FP32)
        nc.vector.tensor_mul(wln, lnw_t, wcopy)
        swln = ep.tile([P, 1], FP32)
        nc.vector.reduce_sum(out=swln, in_=wln, axis=mybir.AxisListType.X)
        scr0 = ep.tile([P, C], FP32)
        c0 = ep.tile([P, 1], FP32)
        nc.vector.tensor_mul(scr0, lnb_t, wcopy)
        nc.vector.reduce_sum(out=c0, in_=scr0, axis=mybir.AxisListType.X)
        nc.vector.tensor_add(c0, c0, b_t)

        # dot = s . wln  (per batch row)
        scr1 = ep.tile([P, C], FP32)
        dot = ep.tile([P, 1], FP32)
        nc.vector.tensor_mul(scr1[:bs], s[:bs], wln[:bs])
        nc.vector.reduce_sum(out=dot[:bs], in_=scr1[:bs], axis=mybir.AxisListType.X)

        # logits = rstd * (dot - mu * swln); out = sigmoid(logits + c0)
        t0 = ep.tile([P, 1], FP32)
        nc.vector.tensor_mul(t0[:bs], mu, swln[:bs])
        t1 = ep.tile([P, 1], FP32)
        nc.vector.tensor_sub(t1[:bs], dot[:bs], t0[:bs])
        t2 = ep.tile([P, 1], FP32)
        nc.vector.tensor_mul(t2[:bs], t1[:bs], rstd[:bs])
        res = ep.tile([P, 1], FP32)
        nc.scalar.activation(
            out=res[:bs],
            in_=t2[:bs],
            func=mybir.ActivationFunctionType.Sigmoid,
            bias=c0[:bs],
            scale=1.0,
        )
        nc.sync.dma_start(out=out[:, :], in_=res[:bs])

    # Run the Bacc compile pipeline (register allocation + multi-sync-wait
    # splitting via generate_event_semaphores) — nothing else in the
    # run_bass_kernel_spmd/axon path calls finalize for us.
    nc.finalize()
    return nc


_NC_CACHE = {}


def kernel(**inputs) -> np.ndarray:
    global LAST_RESULT
    x = np.ascontiguousarray(np.asarray(inputs["x"], dtype=np.float32))
    ln_w = np.ascontiguousarray(np.asarray(inputs["ln_w"], dtype=np.float32))
    ln_b = np.ascontiguousarray(np.asarray(inputs["ln_b"], dtype=np.float32))
    W = np.ascontiguousarray(np.asarray(inputs["W"], dtype=np.float32))
    b = np.ascontiguousarray(np.asarray(inputs["b"], dtype=np.float32))

    if "full" not in _NC_CACHE:
        _NC_CACHE["full"] = build()
    nc = _NC_CACHE["full"]
    mask = make_mask(G_DEFAULT)

    in_maps = [
        {
            "x": x[i * BS : (i + 1) * BS],
            "ln_w": ln_w,
            "ln_b": ln_b,
            "W": W,
            "b": b,
            "mask": mask,
        }
        for i in range(NCORES)
    ]
    res = run_bass_kernel_spmd(nc, in_maps, list(range(NCORES)))
    LAST_RESULT = res
    return np.concatenate([res.results[i]["out"] for i in range(NCORES)], axis=0)
